# revision 22
# baseline (speedup 1.0000x reference)
"""Single-head attention (B=8, S=2048, D=384) on 8 NeuronCores.

Sharding: data-parallel over batch — core b computes batch element b
entirely, weights replicated.

Host-side marshalling (layout/dtype only, zero host FLOPs): x is fed
pre-transposed per core as xT [D, S] in fp16, Wq/Wk natural [e, d] in
fp16, Wv pre-transposed [d, e] in fp16.

Per-core dataflow (all on one NeuronCore, f32 out):
  - G = Wq^T Wk  [d1, d2] on the PE (9 small matmuls that also warm the
    PE p-state while x streams in). scores = x G x^T makes the separate
    Q and K projections unnecessary: one projection TT = G^T xT [d2, S]
    replaces both (saves ~18k PE cycles and the Q/K PSUM->SBUF drains).
  - V = x @ Wv^T in natural [S, D] layout with a ones-column pair
    appended -> vA [S, D+2] (bf16).
  - scores^T tile pa[k, q] = xT-block^T @ TT accumulated over the 3
    d2-tiles; exp() on ScalarE straight off PSUM (|logit| < ~50 so f32
    exp cannot overflow, and bf16 shares the f32 exponent range; softmax
    shift-invariance makes the result match the reference's
    max-subtracted computation up to rounding).
  - out_raw[q, :D] and the softmax denominator accumulate TOGETHER via
    out_acc[q, 0:D+2] += exp[k, q-block]^T @ vA[k-block, :] (the ones
    columns of vA make column D equal sum_k exp) — no cross-partition
    reduction ever needed.
  - out[q, e] = out_raw[q, e] * (1 / out_acc[q, D]), written bf16.

QK-side matmuls run fp16 (same PE rate as f32r, exact fp16 products,
half the DMA/SBUF bytes); the exp/PV path runs bf16 (needs f32 exponent
range — fp16 would overflow at e^50 — and its bf16 stationary loads
hide under the 386-col PV matmuls where f32r ones did not). Measured
rel err 5.6e-3 against the f32 reference, inside the 2e-2 gate.

Scheduling notes (from perfetto traces): dma_start triggers cost ~650ns
of serialized sequencer time each, so operands move as single coarse
[128, 3, N] transfers on ONE priority-ordered sync stream; the PE
p-state ramps 0.65->2.4GHz over ~3us of CONTINUOUS execution (any idle
resets it), so throwaway warm-up matmuls bridge from sequencer-ready
(~8us) until the first operands land (~11.6us); epilogues for chunks
0-2 stay off the ACT engine so EXPs are never delayed.
"""

import numpy as np

import concourse.bacc as bacc
import concourse.tile as tile
from concourse import mybir
from concourse import bass_utils

P = 128          # partitions / PE tile edge
S = 2048         # sequence length per core
D = 384          # model dim
NB = 8           # batch == number of cores
DT = D // P      # 3 feature tiles
ST = S // P      # 16 sequence tiles
QC = 512         # q-column chunk (PSUM bank of f32)
NQ = S // QC     # 4 q chunks
F32 = mybir.dt.float32
F32R = mybir.dt.float32r
F16 = mybir.dt.float16
BF16 = mybir.dt.bfloat16


def _build():
    nc = bacc.Bacc(
        "TRN2", target_bir_lowering=False, debug=False, enable_asserts=False
    )
    xt = nc.dram_tensor("xt", [D, S], F16, kind="ExternalInput").ap()
    wq = nc.dram_tensor("wq", [D, D], F16, kind="ExternalInput").ap()
    wk = nc.dram_tensor("wk", [D, D], F16, kind="ExternalInput").ap()
    wvt = nc.dram_tensor("wvt", [D, D], F16, kind="ExternalInput").ap()
    out = nc.dram_tensor("out", [S, D], BF16, kind="ExternalOutput").ap()

    # [128, DT, *] views so each operand moves in ONE dma_start — the
    # ~650ns per-trigger sequencer cost dominated the old head/tail
    xt_r = xt.rearrange("(t p) s -> p t s", p=P)
    wq_r = wq.rearrange("(t p) d -> p t d", p=P)
    wk_r = wk.rearrange("(t p) d -> p t d", p=P)
    wvt_r = wvt.rearrange("(t p) d -> p t d", p=P)

    with tile.TileContext(nc) as tc:
        with (
            tc.tile_pool(name="const", bufs=1) as const_pool,
            tc.tile_pool(name="big", bufs=1) as big,
            tc.tile_pool(name="expool", bufs=4) as ex_pool,
            tc.tile_pool(name="obpool", bufs=8) as ob_pool,
            tc.tile_pool(name="smalls", bufs=4) as small_pool,
            tc.tile_pool(name="ps_stage", bufs=4, space="PSUM") as ps_stage,
            tc.tile_pool(name="ps_acc", bufs=4, space="PSUM") as ps_acc,
        ):
            ones_c = const_pool.tile([P, 2], F32, tag="ones", name="ones_c")
            nc.vector.memset(ones_c, 1.0)
            scratch = const_pool.tile([P, QC], F16, tag="scr", name="scratch")
            nc.vector.memset(scratch, 0.0)

            # Persistent per-core operands.
            xT = big.tile([P, DT, S], F16, tag="xT", name="xT")
            tt = big.tile([P, DT, S], F16, tag="tt", name="tt")
            # +2 ones columns so the denominator rides along col D (col D+1
            # keeps the free size even)
            vA = big.tile([P, ST, D + 2], BF16, tag="vA", name="vA")
            wqS = big.tile([P, DT, D], F16, tag="wq", name="wqS")
            wkS = big.tile([P, DT, D], F16, tag="wk", name="wkS")
            wvT = big.tile([P, DT, D], F16, tag="wvT", name="wvT")
            g = big.tile([P, DT, D], F16, tag="g", name="g")

            # ---- load operands: ONE sync stream in PE-consumption order.
            # The 16 DMA rings are shared and round-robin across open
            # transfers, so priority comes from enqueue order on a single
            # queue, not from spreading queues. The head is input-bandwidth
            # bound (~2.5MB at ~330GB/s = [8.7,16us]); wq/wk go first so
            # the G build gives the PE real ramp-up work at ~9.3us --------
            nc.sync.dma_start(out=wqS, in_=wq_r)
            nc.sync.dma_start(out=wkS, in_=wk_r)
            nc.sync.dma_start(out=wvT, in_=wvt_r)
            nc.sync.dma_start(
                out=xT[:, :, 0:QC // 2], in_=xt_r[:, :, 0:QC // 2]
            )
            nc.sync.dma_start(
                out=xT[:, :, QC // 2:QC], in_=xt_r[:, :, QC // 2:QC]
            )
            for qc in range(1, NQ):
                nc.sync.dma_start(
                    out=xT[:, :, qc * QC:(qc + 1) * QC],
                    in_=xt_r[:, :, qc * QC:(qc + 1) * QC],
                )

            # ---- PE warm-up: the p-state ramp (0.65->2.4GHz after ~3us of
            # continuous execution) starts ticking on throwaway matmuls
            # that bridge until wq/wk land; any PE idle resets the ramp ---
            warm_ps = ps_stage.tile([P, QC], F32, tag="ps1", name="warm")
            for _ in range(8):
                nc.tensor.matmul(
                    warm_ps, scratch[:, 0:P], scratch, start=True, stop=True
                )

            # ---- projections ---------------------------------------------
            # Rotate staging across BOTH psum pools: during this phase the
            # 4 accumulator banks are idle, and 8 rotating banks let the PE
            # run ahead of the DVE drain instead of stalling on a free slot.
            _proj_n = [0]

            def proj_tile():
                _proj_n[0] += 1
                if _proj_n[0] % 2:
                    return ps_stage.tile([P, QC], F32, tag="ps1", name="pj")
                return ps_acc.tile([P, QC], F32, tag="acc", name="pj")

            def project_v(st):
                # V natural: V[s, e] = sum_d xT[d, s] * WvT[d, e]
                pv = proj_tile()
                for dt_ in range(DT):
                    nc.tensor.matmul(
                        pv[:, 0:D],
                        xT[:, dt_, st * P:(st + 1) * P],
                        wvT[:, dt_, :],
                        start=(dt_ == 0),
                        stop=(dt_ == DT - 1),
                    )
                nc.vector.tensor_copy(vA[:, st, 0:D], pv[:, 0:D])

            def build_g():
                # G[d1, d2] = sum_e Wq[e, d1] * Wk[e, d2]
                for d1t in range(DT):
                    pg = proj_tile()
                    for et in range(DT):
                        nc.tensor.matmul(
                            pg[:, 0:D],
                            wqS[:, et, d1t * P:(d1t + 1) * P],
                            wkS[:, et, :],
                            start=(et == 0),
                            stop=(et == DT - 1),
                        )
                    nc.vector.tensor_copy(g[:, d1t, :], pg[:, 0:D])

            def project_tt(qc):
                # TT[d2, s] = sum_d1 G[d1, d2] * xT[d1, s]
                for d2t in range(DT):
                    pp = proj_tile()
                    for d1t in range(DT):
                        nc.tensor.matmul(
                            pp,
                            g[:, d1t, d2t * P:(d2t + 1) * P],
                            xT[:, d1t, qc * QC:(qc + 1) * QC],
                            start=(d1t == 0),
                            stop=(d1t == DT - 1),
                        )
                    nc.vector.tensor_copy(tt[:, d2t, qc * QC:(qc + 1) * QC], pp)

            # G first (its wq/wk land first and it runs while the PE is
            # still ramping), then per 512-col x chunk: V rows then TT
            # columns — matching the DMA arrival order
            build_g()
            for qc in range(NQ):
                for st in range(qc * 4, qc * 4 + 4):
                    project_v(st)
                project_tt(qc)
            # ones columns for every V row tile in one strided copy
            nc.vector.tensor_copy(
                vA[:, :, D:D + 2],
                ones_c.unsqueeze(1).broadcast_to([P, ST, 2]),
            )

            # ---- attention, one 512-wide q chunk at a time ----------------
            for c in range(NQ):
                accs = [
                    ps_acc.tile([P, D + 2], F32, tag="acc", name="acc")
                    for _ in range(4)
                ]

                def emit_pv(kt_i, ex):
                    for qs in range(4):
                        nc.tensor.matmul(
                            accs[qs],
                            ex[:, qs * P:(qs + 1) * P],
                            vA[:, kt_i, :],
                            start=(kt_i == 0),
                            stop=(kt_i == ST - 1),
                        )

                pending = []
                for kt_i in range(ST):
                    pa = ps_stage.tile([P, QC], F32, tag="ps1", name="pa")
                    for d2t in range(DT):
                        nc.tensor.matmul(
                            pa,
                            xT[:, d2t, kt_i * P:(kt_i + 1) * P],
                            tt[:, d2t, c * QC:(c + 1) * QC],
                            start=(d2t == 0),
                            stop=(d2t == DT - 1),
                        )
                    ex = ex_pool.tile([P, QC], BF16, tag="ex", name="ex")
                    nc.scalar.activation(
                        ex, pa, mybir.ActivationFunctionType.Exp
                    )
                    # software-pipeline PV two k-tiles behind the QK+exp so
                    # the PE never waits on a just-issued exp
                    pending.append((kt_i, ex))
                    if len(pending) > 2:
                        emit_pv(*pending.pop(0))
                for item in pending:
                    emit_pv(*item)

                # epilogue. For chunks 0-2 everything runs on DVE: putting
                # COPYs on ACT delays the next chunk's EXPs (the PE stalls
                # on them), and out-triggers on the scalar queue wedge
                # ~650ns between EXP dispatches. The final chunk has no
                # EXPs left, so it splits across DVE and ACT to halve the
                # tail chain — with both ACT COPYs issued before their
                # triggers (a trigger between COPYs costs 650ns of ACT).
                recs = []
                for qs in range(4):
                    rec = small_pool.tile([P, 1], F32, tag="rec", name="rec")
                    nc.vector.reciprocal(rec, accs[qs][:, D:D + 1])
                    recs.append(rec)
                obs = [
                    ob_pool.tile([P, D], BF16, tag="ob", name="ob")
                    for _ in range(4)
                ]
                last = c == NQ - 1
                for qs in range(4):
                    if last and qs % 2:
                        nc.scalar.activation(
                            obs[qs],
                            accs[qs][:, 0:D],
                            mybir.ActivationFunctionType.Copy,
                            scale=recs[qs],
                        )
                    else:
                        nc.vector.tensor_scalar_mul(
                            obs[qs], accs[qs][:, 0:D], recs[qs]
                        )
                for qs in range(4):
                    qt_row = (c * 4 + qs) * P
                    eng = nc.scalar if (last and qs % 2) else nc.sync
                    eng.dma_start(
                        out=out[qt_row:qt_row + P, :], in_=obs[qs]
                    )

    nc.compile()
    return nc


_NC = None
_FAST = None


def _get_nc():
    global _NC
    if _NC is None:
        _NC = _build()
    return _NC


def _fast_runner():
    """Build (once) a jitted shard_map callable over the 8 cores.

    Mirrors bass2jax.run_bass_via_pjrt's multi-core branch, but keeps the
    jitted function alive across kernel() calls so repeat invocations skip
    re-trace/re-compile.
    """
    global _FAST
    if _FAST is not None:
        return _FAST
    import jax
    from jax.experimental.shard_map import shard_map
    from jax.sharding import Mesh, PartitionSpec

    from concourse import bass2jax

    import jax.numpy as jnp

    nc = _get_nc()
    bass2jax.install_neuronx_cc_hook()

    in_names = ["xt", "wq", "wk", "wvt"]
    out_aval = jax.core.ShapedArray((S, D), jnp.bfloat16)

    def _body(*args):
        operands = list(args)
        operands.append(bass2jax.partition_id_tensor())
        outs = bass2jax._bass_exec_p.bind(
            *operands,
            out_avals=(out_aval,),
            in_names=tuple(in_names) + ("out", "partition_id"),
            out_names=("out",),
            lowering_input_output_aliases=(),
            sim_require_finite=True,
            sim_require_nnan=True,
            nc=nc,
        )
        return tuple(outs)

    devices = jax.devices()[:NB]
    mesh = Mesh(np.asarray(devices), ("core",))
    n_in = len(in_names) + 1  # + donated zero output
    fn = jax.jit(
        shard_map(
            _body,
            mesh=mesh,
            in_specs=(PartitionSpec("core"),) * n_in,
            out_specs=(PartitionSpec("core"),),
            check_rep=False,
        ),
        donate_argnums=(n_in - 1,),
        keep_unused=True,
    )
    _FAST = fn
    return fn


def _marshal(att_input, Wq, Wk, Wv):
    att_input = np.asarray(att_input, dtype=np.float32)
    # pre-transposed per-core x, natural Wq/Wk, transposed Wv — fp16
    # (layout + dtype only, no FLOPs)
    xts = np.ascontiguousarray(
        att_input.transpose(0, 2, 1)
    ).astype(np.float16)  # [NB, D, S]
    wq16 = np.asarray(Wq, dtype=np.float16)
    wk16 = np.asarray(Wk, dtype=np.float16)
    wvt16 = np.ascontiguousarray(
        np.asarray(Wv, dtype=np.float32).T
    ).astype(np.float16)
    return xts, (wq16, wk16, wvt16)


def run(att_input, Wq, Wk, Wv, trace=False):
    xts, wts = _marshal(att_input, Wq, Wk, Wv)
    if trace:
        in_maps = [
            {"xt": xts[b], "wq": wts[0], "wk": wts[1], "wvt": wts[2]}
            for b in range(NB)
        ]
        res = bass_utils.run_bass_kernel_spmd(
            _get_nc(), in_maps, core_ids=list(range(NB)), trace=True
        )
        out = np.stack([res.results[b]["out"] for b in range(NB)], axis=0)
        return out.astype(np.float32, copy=False), res

    try:
        import ml_dtypes

        fn = _fast_runner()
        xs = xts.reshape(NB * D, S)
        ws = [np.concatenate([w] * NB, axis=0) for w in wts]
        zeros = np.zeros((NB * S, D), ml_dtypes.bfloat16)
        (out,) = fn(xs, *ws, zeros)
        out = np.asarray(out)
    except Exception:
        # fallback: the stock SPMD runner (re-jits per call, same NEFF)
        in_maps = [
            {"xt": xts[b], "wq": wts[0], "wk": wts[1], "wvt": wts[2]}
            for b in range(NB)
        ]
        res = bass_utils.run_bass_kernel_spmd(
            _get_nc(), in_maps, core_ids=list(range(NB))
        )
        out = np.stack([res.results[b]["out"] for b in range(NB)], axis=0)
    return out.reshape(NB, S, D).astype(np.float32, copy=False), None


def kernel(att_input, Wq, Wk, Wv):
    out, _ = run(att_input, Wq, Wk, Wv)
    return out
